# revision 14
# baseline (speedup 1.0000x reference)
"""BrainGNN message-passing + GRU cell kernel for 8 TRN2 NeuronCores.

Reference computation (N=16384 nodes, H=32):
    m  = adj @ node_state                      # [N, H]
    x  = m @ Wm.T + bm
    gi = x @ W_ih.T + b_ih ; gh = node_state @ W_hh.T + b_hh
    r = sig(gi_r + gh_r); z = sig(gi_z + gh_z); n = tanh(gi_n + r*gh_n)
    out = (1-z)*n + z*node_state

Sharding: row-shard adj and the output across 8 cores (2048 rows each);
node_state + tiny weights replicated.

This version is memory-roofline driven: the only mandatory HBM traffic is
one pass over adj, so adj is pre-cast to fp16 AND pre-transposed on the
host into a per-core, per-group [col-block partition, k, row] layout.
That:
  - halves HBM traffic vs f32 (64 MiB/core; ~358 GB/s/NC HBM cap
    -> ~187 us DMA floor vs the 429 us f32 baseline),
  - eliminates all on-device PE transposes of adj (half the baseline's
    PE work), keeping PE (~110 us) well under the DMA stream,
  - makes every stream DMA per-partition contiguous (4 KiB runs).
fp16 for adj is numerically identical to the f32 baseline kernel (which
cast f32->fp16 inline during DMA); measured rel err vs f64 ~ 8e-3.
fp8 variants were simulated offline and fail the 2e-2 gate (0.33+).

Weight folding: gi = adj @ sc + bc with sc = node_state @ (W_ih @ Wm).T
precomputed on host ([N, 96], fp16 stationary operand) and
bc = b_ih + bm @ W_ih.T folded into the gate biases.  The single big
accumulating gemm per 512-row group produces all three gate
pre-activations directly ([96, 512] PSUM tile); Wm / W_ih gemms vanish.
gh(r,z) = W_hh(r,z) @ hT rides the same PSUM accumulation group; only
h_n needs its own PSUM tile (n-gate mixes r * h_n before tanh).

Per-core pipeline, per 512-row group (4 groups):
  - 32x 512 KiB HWDGE DMAs stream the group's adj slice (fp16, [128,
    2048] tiles, per-partition contiguous)
  - 129 accumulating PE matmuls (stationary sc k-slice [128, 96] fp16,
    moving adj tile [128, 512] fp16) -> gi(+gh_rz) [96, 512] f32 PSUM
  - GRU gate math in the transposed [feature, row] layout: ACT
    sigmoid/tanh with fused per-partition biases, elementwise mix on
    DVE; PE transpose-mode un-transposes [32, 512] -> 4x [128, 32]
  - one 64 KiB output DMA per group ([p, kr, h] layout, host unshuffles)
Gates of group g overlap the DMA stream of group g+1.
"""

from contextlib import ExitStack
from concurrent.futures import ThreadPoolExecutor

import numpy as np

import concourse.bass as bass
import concourse.mybir as mybir
import concourse.tile as tile
from concourse import bacc
from concourse.bass_utils import run_bass_kernel_spmd

F32 = mybir.dt.float32
F16 = mybir.dt.float16

N_CORES = 8
N_FULL = 16384
H = 32
R = N_FULL // N_CORES   # 2048 rows per core
GR = 512                # rows per group
NG = R // GR            # 4 groups
KB = N_FULL // 128      # 128 contraction k-blocks
KQ = 8                  # k-blocks per stream DMA (1 MiB each)
CHUNK_BUFS = 8
SEQ = True              # adj16 laid out so each stream DMA reads a fully
                        # contiguous block of HBM (chunk-major layout)


def build_module(loop_iters=None, kq=None, bufs=None, seq=None):
    """Per-core Bass module. loop_iters: wrap body in device-side For_i
    (slope-based HW timing only)."""
    kq = KQ if kq is None else kq
    bufs = CHUNK_BUFS if bufs is None else bufs
    seq = SEQ if seq is None else seq
    n_chunks_g = KB // kq
    nc = bacc.Bacc(
        "TRN2", target_bir_lowering=False, debug=False, num_devices=N_CORES
    )
    if seq:
        # adj16: [(g*nchunks + kk)*128 + col-partition, j*GR + row]
        adj16_d = nc.declare_dram_parameter(
            "adj16", [NG * n_chunks_g * 128, kq * GR], F16, isOutput=False)
    else:
        # adj16: [g*128 + col-block-partition, k*GR + row]  (fp16, pre-transposed)
        adj16_d = nc.declare_dram_parameter(
            "adj16", [NG * 128, KB * GR], F16, isOutput=False)
    sc_d = nc.declare_dram_parameter("sc16", [128, KB * 3 * H], F16, isOutput=False)
    statef_d = nc.declare_dram_parameter("statef", [128, (R // 128) * H], F32, isOutput=False)
    whhT_d = nc.declare_dram_parameter("whhT", [H, 3 * H], F32, isOutput=False)
    bias4_d = nc.declare_dram_parameter("bias4", [H, 4], F32, isOutput=False)
    identf_d = nc.declare_dram_parameter("identf", [128, 128], F32, isOutput=False)
    out_d = nc.declare_dram_parameter("out", [128, (R // 128) * H], F32, isOutput=True)

    with tile.TileContext(nc) as tc:
        with (
            tc.tile_pool(name="const", bufs=1) as cpool,
            tc.tile_pool(name="chunks", bufs=bufs) as chpool,
            tc.tile_pool(name="small", bufs=2) as spool,
            tc.tile_pool(name="pgi", bufs=2, space="PSUM") as pgi,
            tc.tile_pool(name="ptp", bufs=2, space="PSUM") as ptp,
            tc.tile_pool(name="pgate", bufs=2, space="PSUM") as pgate,
        ):
            # ---- constants (outside any timing loop) ----
            # On the scalar HWDGE ring so the adj stream (sync ring) starts
            # at t=0 in one-shot runs; sc split into slices so the first
            # body matmuls only wait on slice 0.
            KSL = 16
            sc_tiles = []
            for sl in range(KB // KSL):
                t = cpool.tile([128, KSL * 3 * H], F16, tag=f"sc{sl}")
                nc.scalar.dma_start(
                    out=t[:], in_=sc_d[:, sl * KSL * 3 * H:(sl + 1) * KSL * 3 * H])
                sc_tiles.append(t)

            def sc_slice(k):
                t = sc_tiles[k // KSL]
                j = k % KSL
                return t[:, j * 3 * H:(j + 1) * 3 * H]
            statef_sb = cpool.tile([128, (R // 128) * H], F32, tag="statef")
            nc.scalar.dma_start(out=statef_sb[:], in_=statef_d[:])
            whhT_sb = cpool.tile([H, 3 * H], F32, tag="whhT")
            nc.scalar.dma_start(out=whhT_sb[:], in_=whhT_d[:])
            bias4_sb = cpool.tile([H, 4], F32, tag="bias4")
            nc.scalar.dma_start(out=bias4_sb[:], in_=bias4_d[:])
            identf_sb = cpool.tile([128, 128], F32, tag="identf")
            nc.scalar.dma_start(out=identf_sb[:], in_=identf_d[:])
            ident32f = identf_sb[0:32, 0:32]

            _lctx = ExitStack()
            if loop_iters is not None:
                _lctx.enter_context(tc.For_i(0, loop_iters, 1))
            for g in range(NG):
                # hT = this group's node_state rows, transposed (f32 exact)
                hps = ptp.tile([H, GR], F32, tag="hps")
                for s in range(4):
                    kblk = g * 4 + s
                    nc.tensor.matmul(
                        hps[:, s * 128:(s + 1) * 128],
                        lhsT=statef_sb[:, kblk * H:(kblk + 1) * H],
                        rhs=identf_sb[:],
                        is_transpose=True,
                        start=(s == 0),
                        stop=(s == 3),
                    )
                hT = spool.tile([H, GR], F32, tag="hT")
                nc.vector.tensor_copy(hT[:], hps[:])

                # gi = adj @ sc accumulated over all 128 k-blocks; the
                # gh(r,z) gemm rides the same accumulation group.
                gips = pgi.tile([3 * H, GR], F32, tag="gips")
                for kk in range(n_chunks_g):
                    ch = chpool.tile([128, kq * GR], F16, tag="chunk")
                    if seq:
                        c = g * n_chunks_g + kk
                        src = adj16_d[c * 128:(c + 1) * 128, :]
                    else:
                        src = adj16_d[g * 128:(g + 1) * 128,
                                      kk * kq * GR:(kk + 1) * kq * GR]
                    nc.sync.dma_start(out=ch[:], in_=src)
                    for j in range(kq):
                        k = kk * kq + j
                        if k == KB - 1:
                            nc.tensor.matmul(
                                gips[0:2 * H, :], lhsT=whhT_sb[:, 0:2 * H],
                                rhs=hT[:], start=False, stop=False,
                            )
                        nc.tensor.matmul(
                            gips[:],
                            lhsT=sc_slice(k),
                            rhs=ch[:, j * GR:(j + 1) * GR],
                            start=(k == 0),
                            stop=(k == KB - 1),
                        )

                # h_n separate (n-gate needs r * h_n before tanh)
                hnps = pgate.tile([H, GR], F32, tag="hnps")
                nc.tensor.matmul(
                    hnps[:], lhsT=whhT_sb[:, 2 * H:3 * H], rhs=hT[:],
                    start=True, stop=True,
                )

                # ---- GRU gates (fp32, transposed [feature, row] layout) ----
                r_sb = spool.tile([H, GR], F32, tag="r")
                nc.scalar.activation(
                    r_sb[:], gips[0:H, :], mybir.ActivationFunctionType.Sigmoid,
                    bias=bias4_sb[:, 0:1],
                )
                z_sb = spool.tile([H, GR], F32, tag="z")
                nc.scalar.activation(
                    z_sb[:], gips[H:2 * H, :], mybir.ActivationFunctionType.Sigmoid,
                    bias=bias4_sb[:, 1:2],
                )
                hn_sb = spool.tile([H, GR], F32, tag="hn")
                nc.scalar.activation(
                    hn_sb[:], hnps[:], mybir.ActivationFunctionType.Identity,
                    bias=bias4_sb[:, 3:4],
                )
                rn_sb = spool.tile([H, GR], F32, tag="rn")
                nc.vector.tensor_mul(rn_sb[:], r_sb[:], hn_sb[:])
                rn2_sb = spool.tile([H, GR], F32, tag="rn2")
                nc.vector.tensor_add(rn2_sb[:], rn_sb[:], gips[2 * H:3 * H, :])
                n_sb = spool.tile([H, GR], F32, tag="n")
                nc.scalar.activation(
                    n_sb[:], rn2_sb[:], mybir.ActivationFunctionType.Tanh,
                    bias=bias4_sb[:, 2:3],
                )
                # out = n + z * (h - n)
                d_sb = spool.tile([H, GR], F32, tag="d")
                nc.vector.tensor_sub(d_sb[:], hT[:], n_sb[:])
                zd_sb = spool.tile([H, GR], F32, tag="zd")
                nc.vector.tensor_mul(zd_sb[:], z_sb[:], d_sb[:])
                oT_sb = spool.tile([H, GR], F32, tag="oT")
                nc.vector.tensor_add(oT_sb[:], n_sb[:], zd_sb[:])

                # un-transpose [32, 512] -> 4 x [128, 32]; one DMA per group
                ou_sb = spool.tile([128, 4 * H], F32, tag="ou")
                for s in range(4):
                    ops_t = pgate.tile([128, H], F32, tag="otp")
                    nc.tensor.matmul(
                        ops_t[:],
                        lhsT=oT_sb[:, s * 128:(s + 1) * 128],
                        rhs=ident32f,
                        is_transpose=True,
                        start=True,
                        stop=True,
                    )
                    nc.scalar.copy(ou_sb[:, s * H:(s + 1) * H], ops_t[:])
                # scalar-engine HWDGE ring: keeps this (gate-chain-dependent)
                # store from head-of-line blocking the next group's stream
                # DMAs on the sync ring
                nc.scalar.dma_start(
                    out=out_d[:, g * 4 * H:(g + 1) * 4 * H], in_=ou_sb[:]
                )
            _lctx.close()
    nc.compile()
    return nc


def _prep_adj(adj, kq=None, seq=None):
    """Per-core fp16 arrays, pre-transposed for the stream DMAs.

    seq=False: adj16_c[g*128+p, k*GR+r]              = adj[c*R+g*GR+r, k*128+p]
    seq=True:  adj16_c[(g*ncg+kk)*128+p, j*GR+r]     = adj[c*R+g*GR+r, (kk*kq+j)*128+p]
    (chunk-major: each [128, kq*GR] stream DMA reads contiguous HBM)"""
    kq = KQ if kq is None else kq
    seq = SEQ if seq is None else seq
    ncg = KB // kq
    shape = (NG * ncg * 128, kq * GR) if seq else (NG * 128, KB * GR)
    outs = [np.empty(shape, np.float16) for _ in range(N_CORES)]

    def prep(cg):
        c, g = cg
        r0 = c * R + g * GR
        if seq:
            src = adj[r0:r0 + GR].reshape(GR, ncg, kq, 128).transpose(1, 3, 2, 0)
            dst = outs[c][g * ncg * 128:(g + 1) * ncg * 128].reshape(
                ncg, 128, kq, GR)
        else:
            src = adj[r0:r0 + GR].reshape(GR, KB, 128).transpose(2, 1, 0)
            dst = outs[c][g * 128:(g + 1) * 128].reshape(128, KB, GR)
        np.copyto(dst, src, casting="same_kind")

    with ThreadPoolExecutor(16) as ex:
        list(ex.map(prep, [(c, g) for c in range(N_CORES) for g in range(NG)]))
    return outs


def _prep_small(node_state, Wm, bm, W_ih, W_hh, b_ih, b_hh):
    f = np.float32
    state = np.asarray(node_state, f)
    Wm, bm = np.asarray(Wm, f), np.asarray(bm, f)
    W_ih, W_hh = np.asarray(W_ih, f), np.asarray(W_hh, f)
    b_ih, b_hh = np.asarray(b_ih, f), np.asarray(b_hh, f)

    # folded stationary operand: gi = adj @ sc + bc
    Wc = np.ascontiguousarray((W_ih @ Wm).T)          # [H, 3H]
    sc = (state @ Wc).astype(np.float16)              # [N, 3H]
    sc16 = np.ascontiguousarray(
        sc.reshape(KB, 128, 3 * H).transpose(1, 0, 2)
    ).reshape(128, KB * 3 * H)                        # [p, k*3H+j]

    b_ih_eff = b_ih + bm @ W_ih.T
    bias4 = np.stack(
        [
            b_ih_eff[0:H] + b_hh[0:H],            # r-gate bias
            b_ih_eff[H:2 * H] + b_hh[H:2 * H],    # z-gate bias
            b_ih_eff[2 * H:3 * H],                # i_n bias
            b_hh[2 * H:3 * H],                    # h_n bias
        ],
        axis=1,
    ).astype(f)
    return {
        "sc16": sc16,
        "whhT": np.ascontiguousarray(W_hh.T),
        "bias4": bias4,
        "identf": np.eye(128, dtype=f),
    }


def _prep_statef(node_state):
    f = np.float32
    state = np.asarray(node_state, f)
    return [
        np.ascontiguousarray(
            state[c * R:(c + 1) * R].reshape(R // 128, 128, H).transpose(1, 0, 2)
        ).reshape(128, (R // 128) * H)
        for c in range(N_CORES)
    ]


_NC_CACHE = {}


def _get_module(loop_iters=None, kq=None, bufs=None, seq=None):
    key = (loop_iters, kq, bufs, seq)
    if key not in _NC_CACHE:
        _NC_CACHE[key] = build_module(loop_iters=loop_iters, kq=kq, bufs=bufs,
                                      seq=seq)
    return _NC_CACHE[key]


def make_in_maps(adj, node_state, Wm, bm, W_ih, W_hh, b_ih, b_hh,
                 kq=None, seq=None):
    f = np.float32
    adj = np.asarray(adj, f)
    small = _prep_small(node_state, Wm, bm, W_ih, W_hh, b_ih, b_hh)
    adj16 = _prep_adj(adj, kq=kq, seq=seq)
    statef = _prep_statef(node_state)
    return [
        {"adj16": adj16[c], "statef": statef[c], **small}
        for c in range(N_CORES)
    ]


def gather_out(res):
    return np.concatenate(
        [
            res.results[c]["out"]
            .reshape(128, R // 128, H)
            .transpose(1, 0, 2)
            .reshape(R, H)
            for c in range(N_CORES)
        ],
        axis=0,
    ).astype(np.float32)


def kernel(adj, node_state, Wm, bm, W_ih, W_hh, b_ih, b_hh):
    in_maps = make_in_maps(adj, node_state, Wm, bm, W_ih, W_hh, b_ih, b_hh)
    nc = _get_module(None)
    res = run_bass_kernel_spmd(nc, in_maps, list(range(N_CORES)))
    return gather_out(res)
